# revision 4
# baseline (speedup 1.0000x reference)
"""ConvLSTM (peephole, kernel_size=1) Trainium2 kernel, 8-core tensor-parallel.

Strategy:
  - TP-8 over gate output channels: core j owns channels [128j, 128j+128) of H
    for all 4 gates (512 output channels per core per step).
  - Per step, ONE fused PSUM accumulation computes xg_t + Wh @ h_t for this
    core's 512 channels: lhsT = xT_t / hT k-tiles [128, 64], rhs = weight
    slabs [128, 512] (weight-streaming orientation; batch on PSUM partitions).
  - Split-precision matmuls: W = hi(fp16) + lo(fp16, scaled by 2^11 to stay
    normal); two PSUM banks, recombined as hi + lo * 2^-11. Activations fp16.
    Host-validated rel err ~3.5e-3 vs fp32 reference.
  - h_{t+1} chunk is PE-transposed to [128ch, 64b], AllGathered across the 8
    cores each step (bounce via internal DRAM), giving every core the full
    hT [128, 8k, 64b] for the next step.
  - Gates/elementwise in fp32 on [64, 512]-shaped tiles (batch on partitions).

Self-contained: hardcodes B=64, S=256, H=1024, 8 cores.
"""

import os
import sys

import numpy as np

sys.path.insert(0, "/opt/trn_rl_repo")

import ml_dtypes  # noqa: E402  (after path insert; available in env)

B, S, H = 64, 256, 1024
NCORES = 8
CH = H // NCORES          # 128 output channels per core
KT = H // 128             # 8 contraction k-tiles
NF = 4 * CH               # 512 = 4 gates x 128 channels, psum free dim
LO_SCALE = 2.0 ** 11

LAST_RESULT = None        # BassKernelResults of the most recent run (for test.py)


def _split_w(arr32):
    """fp32 -> (hi fp16, lo fp16 scaled by 2^11)."""
    hi = arr32.astype(np.float16)
    lo = ((arr32 - hi.astype(np.float32)) * LO_SCALE).astype(np.float16)
    return hi, lo


def _build_program():
    import concourse.bass as bass
    import concourse.mybir as mybir
    import concourse.tile as tile
    from concourse import bacc
    from concourse.masks import make_identity

    f16 = mybir.dt.float16
    f32 = mybir.dt.float32
    ACT = mybir.ActivationFunctionType

    nc = bacc.Bacc("TRN2", target_bir_lowering=False, debug=False,
                   enable_asserts=False, num_devices=NCORES)

    xt_d = nc.dram_tensor("xt", (S, 128, KT, B), f16, kind="ExternalInput")
    wrh_d = nc.dram_tensor("wrh", (KT, 128, NF), f16, kind="ExternalInput")
    wrl_d = nc.dram_tensor("wrl", (KT, 128, NF), f16, kind="ExternalInput")
    wxh_d = nc.dram_tensor("wxh", (KT, 128, NF), f16, kind="ExternalInput")
    wxl_d = nc.dram_tensor("wxl", (KT, 128, NF), f16, kind="ExternalInput")
    h0t_d = nc.dram_tensor("h0t", (128, KT, B), f16, kind="ExternalInput")
    c0c_d = nc.dram_tensor("c0c", (B, CH), f32, kind="ExternalInput")
    bias_d = nc.dram_tensor("bias", (B, NF), f32, kind="ExternalInput")
    peep_d = nc.dram_tensor("peep", (3, B, CH), f32, kind="ExternalInput")
    out_d = nc.dram_tensor("out", (S, B, CH), f16, kind="ExternalOutput")

    with tile.TileContext(nc) as tc:
        with (
            tc.tile_pool(name="wpool", bufs=1) as wpool,
            tc.tile_pool(name="cons", bufs=1) as cons,
            tc.tile_pool(name="xin", bufs=4) as xin,
            tc.tile_pool(name="hga", bufs=2) as hga,
            tc.tile_pool(name="gt", bufs=2) as gt,
            tc.tile_pool(name="cst", bufs=2) as cst,
            tc.tile_pool(name="hout", bufs=3) as hout,
            tc.tile_pool(name="pmm", bufs=2, space="PSUM") as pmm,
            tc.tile_pool(name="ptr", bufs=2, space="PSUM") as ptr,
            tc.tile_pool(name="dbb", bufs=2, space="DRAM") as dbb,
        ):
            # --- persistent weights / constants ---
            wrh = wpool.tile([128, KT, NF], f16, tag="wrh")
            wrl = wpool.tile([128, KT, NF], f16, tag="wrl")
            wxh = wpool.tile([128, KT, NF], f16, tag="wxh")
            wxl = wpool.tile([128, KT, NF], f16, tag="wxl")
            nc.sync.dma_start(wrh[:], wrh_d[:].rearrange("k p n -> p k n"))
            nc.sync.dma_start(wrl[:], wrl_d[:].rearrange("k p n -> p k n"))
            nc.sync.dma_start(wxh[:], wxh_d[:].rearrange("k p n -> p k n"))
            nc.sync.dma_start(wxl[:], wxl_d[:].rearrange("k p n -> p k n"))

            bias = cons.tile([B, NF], f32, tag="bias")
            nc.sync.dma_start(bias[:], bias_d[:])
            peep = cons.tile([B, 3, CH], f32, tag="peep")
            nc.sync.dma_start(peep[:], peep_d[:].rearrange("g p c -> p g c"))
            ident = cons.tile([B, B], f16, tag="ident")
            make_identity(nc, ident[:])

            # --- initial state ---
            hbuf = hga.tile([128, KT, B], f16, tag="hbuf")
            nc.sync.dma_start(hbuf[:], h0t_d[:])
            c_prev = cst.tile([B, CH], f32, tag="c")
            nc.sync.dma_start(c_prev[:], c0c_d[:])

            for t in range(S):
                # x slab for this step (prefetched; bufs=4)
                xb = xin.tile([128, KT, B], f16, tag="xb")
                nc.sync.dma_start(xb[:], xt_d[t])

                ps_hi = pmm.tile([B, NF], f32, tag="ps_hi")
                ps_lo = pmm.tile([B, NF], f32, tag="ps_lo")
                # x-side matmuls first (no dependence on gathered h)
                for k in range(KT):
                    nc.tensor.matmul(ps_hi[:], xb[:, k, :], wxh[:, k, :],
                                     start=(k == 0), stop=False)
                for k in range(KT):
                    nc.tensor.matmul(ps_lo[:], xb[:, k, :], wxl[:, k, :],
                                     start=(k == 0), stop=False)
                # h-side matmuls
                for k in range(KT):
                    nc.tensor.matmul(ps_hi[:], hbuf[:, k, :], wrh[:, k, :],
                                     start=False, stop=(k == KT - 1))
                for k in range(KT):
                    nc.tensor.matmul(ps_lo[:], hbuf[:, k, :], wrl[:, k, :],
                                     start=False, stop=(k == KT - 1))

                # acc = ps_hi + 2^-11 * ps_lo + bias   (gate preacts, [64, 512])
                tlo = gt.tile([B, NF], f32, tag="tlo")
                nc.scalar.activation(tlo[:], ps_lo[:], ACT.Copy,
                                     scale=1.0 / LO_SCALE)
                acc0 = gt.tile([B, NF], f32, tag="acc0")
                nc.vector.tensor_add(acc0[:], ps_hi[:], tlo[:])
                acc = gt.tile([B, NF], f32, tag="acc")
                nc.vector.tensor_add(acc[:], acc0[:], bias[:])

                # gate order in free dim: [i | f | ctil | o] (g-major, 128 each)
                pi = gt.tile([B, CH], f32, tag="pi")
                nc.vector.tensor_mul(pi[:], c_prev[:], peep[:, 0, :])
                pf = gt.tile([B, CH], f32, tag="pf")
                nc.vector.tensor_mul(pf[:], c_prev[:], peep[:, 1, :])
                pre_if = gt.tile([B, 2 * CH], f32, tag="pre_if")
                nc.vector.tensor_add(pre_if[:, 0:CH], acc[:, 0:CH], pi[:])
                nc.vector.tensor_add(pre_if[:, CH:2 * CH], acc[:, CH:2 * CH], pf[:])
                sif = gt.tile([B, 2 * CH], f32, tag="sif")
                nc.scalar.activation(sif[:], pre_if[:], ACT.Sigmoid)
                ctil = gt.tile([B, CH], f32, tag="ctil")
                nc.scalar.activation(ctil[:], acc[:, 2 * CH:3 * CH], ACT.Tanh)

                # c_new = f*c + i + ctil
                fc = gt.tile([B, CH], f32, tag="fc")
                nc.vector.tensor_mul(fc[:], sif[:, CH:2 * CH], c_prev[:])
                ic = gt.tile([B, CH], f32, tag="ic")
                nc.vector.tensor_add(ic[:], sif[:, 0:CH], ctil[:])
                c_new = cst.tile([B, CH], f32, tag="c")
                nc.vector.tensor_add(c_new[:], fc[:], ic[:])

                # o = sigmoid(acc_o + peep_o * c_new); h = o * tanh(c_new)
                po = gt.tile([B, CH], f32, tag="po")
                nc.vector.tensor_mul(po[:], c_new[:], peep[:, 2, :])
                preo = gt.tile([B, CH], f32, tag="preo")
                nc.vector.tensor_add(preo[:], acc[:, 3 * CH:4 * CH], po[:])
                og = gt.tile([B, CH], f32, tag="og")
                nc.scalar.activation(og[:], preo[:], ACT.Sigmoid)
                th = gt.tile([B, CH], f32, tag="th")
                nc.scalar.activation(th[:], c_new[:], ACT.Tanh)
                hsb = hout.tile([B, CH], f16, tag="hsb")
                nc.vector.tensor_mul(hsb[:], og[:], th[:])

                # output chunk for this step (host re-assembles/casts)
                nc.sync.dma_start(out_d[t], hsb[:])

                # transpose h chunk -> [128ch, 64b] and AllGather across cores
                pt_t = ptr.tile([CH, B], f16, tag="pt")
                nc.tensor.transpose(pt_t[:], hsb[:], ident[:])
                htr = hout.tile([CH, B], f16, tag="htr")
                nc.vector.tensor_copy(htr[:], pt_t[:])
                inb = dbb.tile([CH, B], f16, tag="inb")
                nc.sync.dma_start(inb[:], htr[:])
                outb = dbb.tile([H, B], f16, tag="outb")
                nc.gpsimd.collective_compute(
                    "AllGather",
                    mybir.AluOpType.bypass,
                    replica_groups=[list(range(NCORES))],
                    ins=[inb[:].opt()],
                    outs=[outb[:].opt()],
                )
                hbuf = hga.tile([128, KT, B], f16, tag="hbuf")
                nc.sync.dma_start(
                    hbuf[:], outb[:].rearrange("(k p) b -> p k b", p=128))

                c_prev = c_new

    nc.compile()
    return nc


_NC_CACHE = None


def kernel(x, h0, c0, Wx, bx, Wh, bh, peep, bgate):
    global LAST_RESULT, _NC_CACHE
    from concourse import bass_utils

    x = np.asarray(x, dtype=np.float32)
    h0 = np.asarray(h0, dtype=np.float32)
    c0 = np.asarray(c0, dtype=np.float32)
    Wx = np.asarray(Wx, dtype=np.float32)
    Wh = np.asarray(Wh, dtype=np.float32)
    bx = np.asarray(bx, dtype=np.float32)
    bh = np.asarray(bh, dtype=np.float32)
    peep = np.asarray(peep, dtype=np.float32)
    bgate = np.asarray(bgate, dtype=np.float32)

    # ---- host-side input prep ----
    # xT slab: (S, 128p, 8k, 64b); element = x[b, t, 128k+p]
    xt = np.ascontiguousarray(
        x.transpose(1, 2, 0).reshape(S, KT, 128, B).transpose(0, 2, 1, 3)
    ).astype(np.float16)
    # h0T image: (128p, 8k, 64b)
    h0t = np.ascontiguousarray(
        h0.T.reshape(KT, 128, B).transpose(1, 0, 2)).astype(np.float16)
    btot = (bx + bh + bgate)  # (4, H)

    in_maps = []
    for j in range(NCORES):
        lo_c, hi_c = j * CH, (j + 1) * CH
        # weight slabs: arr[k, p, (g,ch)] = W[g, j*128+ch, 128k+p]
        wr = np.ascontiguousarray(
            Wh[:, lo_c:hi_c, :].transpose(2, 0, 1).reshape(KT, 128, NF))
        wx_ = np.ascontiguousarray(
            Wx[:, lo_c:hi_c, :].transpose(2, 0, 1).reshape(KT, 128, NF))
        wrh, wrl = _split_w(wr)
        wxh, wxl = _split_w(wx_)
        bias_j = np.ascontiguousarray(
            np.broadcast_to(btot[:, lo_c:hi_c].reshape(NF), (B, NF))
        ).astype(np.float32)
        peep_j = np.ascontiguousarray(
            np.broadcast_to(peep[:, lo_c:hi_c][:, None, :], (3, B, CH))
        ).astype(np.float32)
        in_maps.append({
            "xt": xt, "wrh": wrh, "wrl": wrl, "wxh": wxh, "wxl": wxl,
            "h0t": h0t, "c0c": np.ascontiguousarray(c0[:, lo_c:hi_c]),
            "bias": bias_j, "peep": peep_j,
        })

    if _NC_CACHE is None:
        _NC_CACHE = _build_program()
    nc = _NC_CACHE

    # NTFF hook (antenv.axon_hooks) is absent in some containers; force the
    # plain execute path so kernel() never crashes on the profiling import.
    prev = os.environ.get("BASS_NEVER_TRACE")
    os.environ["BASS_NEVER_TRACE"] = "1"
    try:
        res = bass_utils.run_bass_kernel_spmd(
            nc, in_maps, core_ids=list(range(NCORES)))
    finally:
        if prev is None:
            os.environ.pop("BASS_NEVER_TRACE", None)
        else:
            os.environ["BASS_NEVER_TRACE"] = prev
    LAST_RESULT = res

    # ---- assemble full output: res[j]["out"] is (S, B, CH) fp16 ----
    chunks = [r["out"].astype(np.float32) for r in res.results]  # list of (S,B,CH)
    full = np.stack(chunks, axis=0)          # (8, S, B, CH)
    full = full.transpose(2, 1, 0, 3).reshape(B, S, H)
    return np.ascontiguousarray(full)


if __name__ == "__main__":
    # smoke: build only
    prog = _build_program()
    print("build ok")


# revision 5
# speedup vs baseline: 1.0012x; 1.0012x over previous
"""ConvLSTM (peephole, kernel_size=1) Trainium2 kernel, 8-core tensor-parallel.

Strategy:
  - TP-8 over gate output channels: core j owns channels [128j, 128j+128) of H
    for all 4 gates (512 output channels per core per step).
  - Per step, ONE fused PSUM accumulation computes xg_t + Wh @ h_t for this
    core's 512 channels: lhsT = xT_t / hT k-tiles [128, 64], rhs = weight
    slabs [128, 512] (weight-streaming orientation; batch on PSUM partitions).
  - Split-precision matmuls: W = hi(fp16) + lo(fp16, scaled by 2^11 to stay
    normal); two PSUM banks, recombined as hi + lo * 2^-11. Activations fp16.
    Host-validated rel err ~3.5e-3 vs fp32 reference.
  - h_{t+1} chunk is PE-transposed to [128ch, 64b], AllGathered across the 8
    cores each step (bounce via internal DRAM), giving every core the full
    hT [128, 8k, 64b] for the next step.
  - Gates/elementwise in fp32 on [64, 512]-shaped tiles (batch on partitions).

Self-contained: hardcodes B=64, S=256, H=1024, 8 cores.
"""

import os
import sys

import numpy as np

sys.path.insert(0, "/opt/trn_rl_repo")

import ml_dtypes  # noqa: E402  (after path insert; available in env)

B, S, H = 64, 256, 1024
NCORES = 8
CH = H // NCORES          # 128 output channels per core
KT = H // 128             # 8 contraction k-tiles
NF = 4 * CH               # 512 = 4 gates x 128 channels, psum free dim
LO_SCALE = 2.0 ** 11

LAST_RESULT = None        # BassKernelResults of the most recent run (for test.py)


def _split_w(arr32):
    """fp32 -> (hi fp16, lo fp16 scaled by 2^11)."""
    hi = arr32.astype(np.float16)
    lo = ((arr32 - hi.astype(np.float32)) * LO_SCALE).astype(np.float16)
    return hi, lo


def _build_program():
    import concourse.bass as bass
    import concourse.mybir as mybir
    import concourse.tile as tile
    from concourse import bacc
    from concourse.masks import make_identity

    f16 = mybir.dt.float16
    f32 = mybir.dt.float32
    ACT = mybir.ActivationFunctionType

    nc = bacc.Bacc("TRN2", target_bir_lowering=False, debug=False,
                   enable_asserts=False, num_devices=NCORES)

    xt_d = nc.dram_tensor("xt", (S, 128, KT, B), f16, kind="ExternalInput")
    wrh_d = nc.dram_tensor("wrh", (KT, 128, NF), f16, kind="ExternalInput")
    wrl_d = nc.dram_tensor("wrl", (KT, 128, NF), f16, kind="ExternalInput")
    wxh_d = nc.dram_tensor("wxh", (KT, 128, NF), f16, kind="ExternalInput")
    wxl_d = nc.dram_tensor("wxl", (KT, 128, NF), f16, kind="ExternalInput")
    h0t_d = nc.dram_tensor("h0t", (128, KT, B), f16, kind="ExternalInput")
    c0c_d = nc.dram_tensor("c0c", (B, CH), f32, kind="ExternalInput")
    bias_d = nc.dram_tensor("bias", (B, NF), f32, kind="ExternalInput")
    peep_d = nc.dram_tensor("peep", (3, B, CH), f32, kind="ExternalInput")
    out_d = nc.dram_tensor("out", (S, B, CH), f16, kind="ExternalOutput")

    with tile.TileContext(nc) as tc:
        with (
            tc.tile_pool(name="wpool", bufs=1) as wpool,
            tc.tile_pool(name="cons", bufs=1) as cons,
            tc.tile_pool(name="xin", bufs=4) as xin,
            tc.tile_pool(name="hga", bufs=2) as hga,
            tc.tile_pool(name="gt", bufs=2) as gt,
            tc.tile_pool(name="cst", bufs=2) as cst,
            tc.tile_pool(name="hout", bufs=3) as hout,
            tc.tile_pool(name="pmm", bufs=2, space="PSUM") as pmm,
            tc.tile_pool(name="ptr", bufs=2, space="PSUM") as ptr,
            tc.tile_pool(name="dbb", bufs=2, space="DRAM") as dbb,
        ):
            # --- persistent weights / constants ---
            wrh = wpool.tile([128, KT, NF], f16, tag="wrh")
            wrl = wpool.tile([128, KT, NF], f16, tag="wrl")
            wxh = wpool.tile([128, KT, NF], f16, tag="wxh")
            wxl = wpool.tile([128, KT, NF], f16, tag="wxl")
            nc.sync.dma_start(wrh[:], wrh_d[:].rearrange("k p n -> p k n"))
            nc.sync.dma_start(wrl[:], wrl_d[:].rearrange("k p n -> p k n"))
            nc.sync.dma_start(wxh[:], wxh_d[:].rearrange("k p n -> p k n"))
            nc.sync.dma_start(wxl[:], wxl_d[:].rearrange("k p n -> p k n"))

            bias = cons.tile([B, NF], f32, tag="bias")
            nc.sync.dma_start(bias[:], bias_d[:])
            peep = cons.tile([B, 3, CH], f32, tag="peep")
            nc.sync.dma_start(peep[:], peep_d[:].rearrange("g p c -> p g c"))
            ident = cons.tile([B, B], f16, tag="ident")
            make_identity(nc, ident[:])

            # --- initial state ---
            hbuf = hga.tile([128, KT, B], f16, tag="hbuf")
            nc.sync.dma_start(hbuf[:], h0t_d[:])
            c_prev = cst.tile([B, CH], f32, tag="c")
            nc.sync.dma_start(c_prev[:], c0c_d[:])

            for t in range(S):
                # x slab for this step (prefetched; bufs=4)
                xb = xin.tile([128, KT, B], f16, tag="xb")
                nc.sync.dma_start(xb[:], xt_d[t])

                ps_hi = pmm.tile([B, NF], f32, tag="ps_hi")
                ps_lo = pmm.tile([B, NF], f32, tag="ps_lo")
                # x-side matmuls first (no dependence on gathered h)
                for k in range(KT):
                    nc.tensor.matmul(ps_hi[:], xb[:, k, :], wxh[:, k, :],
                                     start=(k == 0), stop=False)
                for k in range(KT):
                    nc.tensor.matmul(ps_lo[:], xb[:, k, :], wxl[:, k, :],
                                     start=(k == 0), stop=False)
                # h-side matmuls
                for k in range(KT):
                    nc.tensor.matmul(ps_hi[:], hbuf[:, k, :], wrh[:, k, :],
                                     start=False, stop=(k == KT - 1))
                for k in range(KT):
                    nc.tensor.matmul(ps_lo[:], hbuf[:, k, :], wrl[:, k, :],
                                     start=False, stop=(k == KT - 1))

                # acc = ps_hi + 2^-11 * ps_lo + bias   (gate preacts, [64, 512])
                tlo = gt.tile([B, NF], f32, tag="tlo")
                nc.scalar.activation(tlo[:], ps_lo[:], ACT.Copy,
                                     scale=1.0 / LO_SCALE)
                acc0 = gt.tile([B, NF], f32, tag="acc0")
                nc.vector.tensor_add(acc0[:], ps_hi[:], tlo[:])
                acc = gt.tile([B, NF], f32, tag="acc")
                nc.vector.tensor_add(acc[:], acc0[:], bias[:])

                # gate order in free dim: [i | f | ctil | o] (g-major, 128 each)
                pi = gt.tile([B, CH], f32, tag="pi")
                nc.vector.tensor_mul(pi[:], c_prev[:], peep[:, 0, :])
                pf = gt.tile([B, CH], f32, tag="pf")
                nc.vector.tensor_mul(pf[:], c_prev[:], peep[:, 1, :])
                pre_if = gt.tile([B, 2 * CH], f32, tag="pre_if")
                nc.vector.tensor_add(pre_if[:, 0:CH], acc[:, 0:CH], pi[:])
                nc.vector.tensor_add(pre_if[:, CH:2 * CH], acc[:, CH:2 * CH], pf[:])
                sif = gt.tile([B, 2 * CH], f32, tag="sif")
                nc.scalar.activation(sif[:], pre_if[:], ACT.Sigmoid)
                ctil = gt.tile([B, CH], f32, tag="ctil")
                nc.scalar.activation(ctil[:], acc[:, 2 * CH:3 * CH], ACT.Tanh)

                # c_new = f*c + i + ctil
                fc = gt.tile([B, CH], f32, tag="fc")
                nc.vector.tensor_mul(fc[:], sif[:, CH:2 * CH], c_prev[:])
                ic = gt.tile([B, CH], f32, tag="ic")
                nc.vector.tensor_add(ic[:], sif[:, 0:CH], ctil[:])
                c_new = cst.tile([B, CH], f32, tag="c")
                nc.vector.tensor_add(c_new[:], fc[:], ic[:])

                # o = sigmoid(acc_o + peep_o * c_new); h = o * tanh(c_new)
                po = gt.tile([B, CH], f32, tag="po")
                nc.vector.tensor_mul(po[:], c_new[:], peep[:, 2, :])
                preo = gt.tile([B, CH], f32, tag="preo")
                nc.vector.tensor_add(preo[:], acc[:, 3 * CH:4 * CH], po[:])
                og = gt.tile([B, CH], f32, tag="og")
                nc.scalar.activation(og[:], preo[:], ACT.Sigmoid)
                th = gt.tile([B, CH], f32, tag="th")
                nc.scalar.activation(th[:], c_new[:], ACT.Tanh)
                hsb = hout.tile([B, CH], f16, tag="hsb")
                nc.vector.tensor_mul(hsb[:], og[:], th[:])

                # output chunk for this step (host re-assembles/casts)
                nc.sync.dma_start(out_d[t], hsb[:])

                # transpose h chunk -> [128ch, 64b] and AllGather across cores
                pt_t = ptr.tile([CH, B], f16, tag="pt")
                nc.tensor.transpose(pt_t[:], hsb[:], ident[:])
                htr = hout.tile([CH, B], f16, tag="htr")
                nc.vector.tensor_copy(htr[:], pt_t[:])
                exch = os.environ.get("K_EXCH", "ag")
                gath = os.environ.get("K_GATHER", "one")
                inb = dbb.tile([CH, B], f16, tag="inb")
                nc.sync.dma_start(inb[:], htr[:])
                outb = dbb.tile([H, B], f16, tag="outb")
                if exch == "ag":
                    nc.gpsimd.collective_compute(
                        "AllGather",
                        mybir.AluOpType.bypass,
                        replica_groups=[list(range(NCORES))],
                        ins=[inb[:].opt()],
                        outs=[outb[:].opt()],
                    )
                else:  # diagnostic: fake the gather with 8 local copies
                    for r in range(NCORES):
                        nc.sync.dma_start(outb[r * CH:(r + 1) * CH, :], inb[:])
                hbuf = hga.tile([128, KT, B], f16, tag="hbuf")
                if gath == "one":
                    nc.sync.dma_start(
                        hbuf[:], outb[:].rearrange("(k p) b -> p k b", p=128))
                else:  # split into per-k DMAs (parallel queues)
                    for k in range(KT):
                        nc.sync.dma_start(
                            hbuf[:, k, :], outb[k * 128:(k + 1) * 128, :])

                c_prev = c_new

    nc.compile()
    return nc


_NC_CACHE = None


def kernel(x, h0, c0, Wx, bx, Wh, bh, peep, bgate):
    global LAST_RESULT, _NC_CACHE
    from concourse import bass_utils

    x = np.asarray(x, dtype=np.float32)
    h0 = np.asarray(h0, dtype=np.float32)
    c0 = np.asarray(c0, dtype=np.float32)
    Wx = np.asarray(Wx, dtype=np.float32)
    Wh = np.asarray(Wh, dtype=np.float32)
    bx = np.asarray(bx, dtype=np.float32)
    bh = np.asarray(bh, dtype=np.float32)
    peep = np.asarray(peep, dtype=np.float32)
    bgate = np.asarray(bgate, dtype=np.float32)

    # ---- host-side input prep ----
    # xT slab: (S, 128p, 8k, 64b); element = x[b, t, 128k+p]
    xt = np.ascontiguousarray(
        x.transpose(1, 2, 0).reshape(S, KT, 128, B).transpose(0, 2, 1, 3)
    ).astype(np.float16)
    # h0T image: (128p, 8k, 64b)
    h0t = np.ascontiguousarray(
        h0.T.reshape(KT, 128, B).transpose(1, 0, 2)).astype(np.float16)
    btot = (bx + bh + bgate)  # (4, H)

    in_maps = []
    for j in range(NCORES):
        lo_c, hi_c = j * CH, (j + 1) * CH
        # weight slabs: arr[k, p, (g,ch)] = W[g, j*128+ch, 128k+p]
        wr = np.ascontiguousarray(
            Wh[:, lo_c:hi_c, :].transpose(2, 0, 1).reshape(KT, 128, NF))
        wx_ = np.ascontiguousarray(
            Wx[:, lo_c:hi_c, :].transpose(2, 0, 1).reshape(KT, 128, NF))
        wrh, wrl = _split_w(wr)
        wxh, wxl = _split_w(wx_)
        bias_j = np.ascontiguousarray(
            np.broadcast_to(btot[:, lo_c:hi_c].reshape(NF), (B, NF))
        ).astype(np.float32)
        peep_j = np.ascontiguousarray(
            np.broadcast_to(peep[:, lo_c:hi_c][:, None, :], (3, B, CH))
        ).astype(np.float32)
        in_maps.append({
            "xt": xt, "wrh": wrh, "wrl": wrl, "wxh": wxh, "wxl": wxl,
            "h0t": h0t, "c0c": np.ascontiguousarray(c0[:, lo_c:hi_c]),
            "bias": bias_j, "peep": peep_j,
        })

    if _NC_CACHE is None:
        _NC_CACHE = _build_program()
    nc = _NC_CACHE

    # NTFF hook (antenv.axon_hooks) is absent in some containers; force the
    # plain execute path so kernel() never crashes on the profiling import.
    prev = os.environ.get("BASS_NEVER_TRACE")
    os.environ["BASS_NEVER_TRACE"] = "1"
    try:
        res = bass_utils.run_bass_kernel_spmd(
            nc, in_maps, core_ids=list(range(NCORES)))
    finally:
        if prev is None:
            os.environ.pop("BASS_NEVER_TRACE", None)
        else:
            os.environ["BASS_NEVER_TRACE"] = prev
    LAST_RESULT = res

    # ---- assemble full output: res[j]["out"] is (S, B, CH) fp16 ----
    chunks = [r["out"].astype(np.float32) for r in res.results]  # list of (S,B,CH)
    full = np.stack(chunks, axis=0)          # (8, S, B, CH)
    full = full.transpose(2, 1, 0, 3).reshape(B, S, H)
    return np.ascontiguousarray(full)


if __name__ == "__main__":
    # smoke: build only
    prog = _build_program()
    print("build ok")


# revision 6
# speedup vs baseline: 1.0565x; 1.0553x over previous
"""ConvLSTM (peephole, kernel_size=1) Trainium2 kernel, 8-core tensor-parallel.

Strategy:
  - TP-8 over gate output channels: core j owns channels [128j, 128j+128) of H
    for all 4 gates (512 output channels per core per step).
  - Per step, ONE fused PSUM accumulation computes xg_t + Wh @ h_t for this
    core's 512 channels: lhsT = xT_t / hT k-tiles [128, 64], rhs = weight
    slabs [128, 512] (weight-streaming orientation; batch on PSUM partitions).
  - Split-precision matmuls: W = hi(fp16) + lo(fp16, scaled by 2^11 to stay
    normal); two PSUM banks, recombined as hi + lo * 2^-11. Activations fp16.
    Host-validated rel err ~3.5e-3 vs fp32 reference.
  - h_{t+1} chunk is PE-transposed to [128ch, 64b], AllGathered across the 8
    cores each step (bounce via internal DRAM), giving every core the full
    hT [128, 8k, 64b] for the next step.
  - Gates/elementwise in fp32 on [64, 512]-shaped tiles (batch on partitions).

Self-contained: hardcodes B=64, S=256, H=1024, 8 cores.
"""

import os
import sys

import numpy as np

sys.path.insert(0, "/opt/trn_rl_repo")

import ml_dtypes  # noqa: E402  (after path insert; available in env)

B, S, H = 64, 256, 1024
NCORES = 8
CH = H // NCORES          # 128 output channels per core
KT = H // 128             # 8 contraction k-tiles
NF = 4 * CH               # 512 = 4 gates x 128 channels, psum free dim
LO_SCALE = 2.0 ** 11

LAST_RESULT = None        # BassKernelResults of the most recent run (for test.py)


def _split_w(arr32):
    """fp32 -> (hi fp16, lo fp16 scaled by 2^11)."""
    hi = arr32.astype(np.float16)
    lo = ((arr32 - hi.astype(np.float32)) * LO_SCALE).astype(np.float16)
    return hi, lo


def _build_program():
    import concourse.bass as bass
    import concourse.mybir as mybir
    import concourse.tile as tile
    from concourse import bacc
    from concourse.masks import make_identity

    f16 = mybir.dt.float16
    f32 = mybir.dt.float32
    ACT = mybir.ActivationFunctionType

    nc = bacc.Bacc("TRN2", target_bir_lowering=False, debug=False,
                   enable_asserts=False, num_devices=NCORES)

    xt_d = nc.dram_tensor("xt", (S, 128, KT, B), f16, kind="ExternalInput")
    wrh_d = nc.dram_tensor("wrh", (KT, 128, NF), f16, kind="ExternalInput")
    wrl_d = nc.dram_tensor("wrl", (KT, 128, NF), f16, kind="ExternalInput")
    wxh_d = nc.dram_tensor("wxh", (KT, 128, NF), f16, kind="ExternalInput")
    wxl_d = nc.dram_tensor("wxl", (KT, 128, NF), f16, kind="ExternalInput")
    h0t_d = nc.dram_tensor("h0t", (128, KT, B), f16, kind="ExternalInput")
    c0c_d = nc.dram_tensor("c0c", (B, CH), f32, kind="ExternalInput")
    bias_d = nc.dram_tensor("bias", (B, NF), f32, kind="ExternalInput")
    peep_d = nc.dram_tensor("peep", (3, B, CH), f32, kind="ExternalInput")
    out_d = nc.dram_tensor("out", (S, B, CH), f16, kind="ExternalOutput")

    with tile.TileContext(nc) as tc:
        with (
            tc.tile_pool(name="wpool", bufs=1) as wpool,
            tc.tile_pool(name="cons", bufs=1) as cons,
            tc.tile_pool(name="xin", bufs=4) as xin,
            tc.tile_pool(name="hga", bufs=2) as hga,
            tc.tile_pool(name="gt", bufs=2) as gt,
            tc.tile_pool(name="cst", bufs=2) as cst,
            tc.tile_pool(name="hout", bufs=3) as hout,
            tc.tile_pool(name="pmm", bufs=2, space="PSUM") as pmm,
            tc.tile_pool(name="ptr", bufs=2, space="PSUM") as ptr,
            tc.tile_pool(name="dbb", bufs=2, space="DRAM") as dbb,
        ):
            # --- persistent weights / constants ---
            wrh = wpool.tile([128, KT, NF], f16, tag="wrh")
            wrl = wpool.tile([128, KT, NF], f16, tag="wrl")
            wxh = wpool.tile([128, KT, NF], f16, tag="wxh")
            wxl = wpool.tile([128, KT, NF], f16, tag="wxl")
            nc.sync.dma_start(wrh[:], wrh_d[:].rearrange("k p n -> p k n"))
            nc.sync.dma_start(wrl[:], wrl_d[:].rearrange("k p n -> p k n"))
            nc.sync.dma_start(wxh[:], wxh_d[:].rearrange("k p n -> p k n"))
            nc.sync.dma_start(wxl[:], wxl_d[:].rearrange("k p n -> p k n"))

            bias = cons.tile([B, NF], f32, tag="bias")
            nc.sync.dma_start(bias[:], bias_d[:])
            peep = cons.tile([B, 3, CH], f32, tag="peep")
            nc.sync.dma_start(peep[:], peep_d[:].rearrange("g p c -> p g c"))
            ident = cons.tile([B, B], f16, tag="ident")
            make_identity(nc, ident[:])

            # --- initial state ---
            hbuf = hga.tile([128, KT, B], f16, tag="hbuf")
            nc.sync.dma_start(hbuf[:], h0t_d[:])
            c_prev = cst.tile([B, CH], f32, tag="c")
            nc.sync.dma_start(c_prev[:], c0c_d[:])

            for t in range(S):
                # x slab for this step (prefetched; bufs=4)
                xb = xin.tile([128, KT, B], f16, tag="xb")
                nc.sync.dma_start(xb[:], xt_d[t])

                ps_hi = pmm.tile([B, NF], f32, tag="ps_hi")
                ps_lo = pmm.tile([B, NF], f32, tag="ps_lo")
                # x-side matmuls first (no dependence on gathered h)
                for k in range(KT):
                    nc.tensor.matmul(ps_hi[:], xb[:, k, :], wxh[:, k, :],
                                     start=(k == 0), stop=False)
                for k in range(KT):
                    nc.tensor.matmul(ps_lo[:], xb[:, k, :], wxl[:, k, :],
                                     start=(k == 0), stop=False)
                # h-side matmuls
                for k in range(KT):
                    nc.tensor.matmul(ps_hi[:], hbuf[:, k, :], wrh[:, k, :],
                                     start=False, stop=(k == KT - 1))
                for k in range(KT):
                    nc.tensor.matmul(ps_lo[:], hbuf[:, k, :], wrl[:, k, :],
                                     start=False, stop=(k == KT - 1))

                if os.environ.get("K_GATES", "1") == "0":
                    # diagnostic: matmuls only; h stays stale
                    hsb0 = hout.tile([B, CH], f16, tag="hsb")
                    nc.scalar.activation(hsb0[:], ps_hi[:, 0:CH],
                                         ACT.Copy)
                    nc.scalar.activation(hsb0[:], ps_lo[:, 0:CH],
                                         ACT.Copy)
                    nc.sync.dma_start(out_d[t], hsb0[:])
                    continue
                # acc = ps_hi + 2^-11 * ps_lo + bias   (gate preacts, [64, 512])
                tlo = gt.tile([B, NF], f32, tag="tlo")
                nc.scalar.activation(tlo[:], ps_lo[:], ACT.Copy,
                                     scale=1.0 / LO_SCALE)
                acc0 = gt.tile([B, NF], f32, tag="acc0")
                nc.vector.tensor_add(acc0[:], ps_hi[:], tlo[:])
                acc = gt.tile([B, NF], f32, tag="acc")
                nc.vector.tensor_add(acc[:], acc0[:], bias[:])

                # gate order in free dim: [i | f | ctil | o] (g-major, 128 each)
                pi = gt.tile([B, CH], f32, tag="pi")
                nc.vector.tensor_mul(pi[:], c_prev[:], peep[:, 0, :])
                pf = gt.tile([B, CH], f32, tag="pf")
                nc.vector.tensor_mul(pf[:], c_prev[:], peep[:, 1, :])
                pre_if = gt.tile([B, 2 * CH], f32, tag="pre_if")
                nc.vector.tensor_add(pre_if[:, 0:CH], acc[:, 0:CH], pi[:])
                nc.vector.tensor_add(pre_if[:, CH:2 * CH], acc[:, CH:2 * CH], pf[:])
                sif = gt.tile([B, 2 * CH], f32, tag="sif")
                nc.scalar.activation(sif[:], pre_if[:], ACT.Sigmoid)
                ctil = gt.tile([B, CH], f32, tag="ctil")
                nc.scalar.activation(ctil[:], acc[:, 2 * CH:3 * CH], ACT.Tanh)

                # c_new = f*c + i + ctil
                fc = gt.tile([B, CH], f32, tag="fc")
                nc.vector.tensor_mul(fc[:], sif[:, CH:2 * CH], c_prev[:])
                ic = gt.tile([B, CH], f32, tag="ic")
                nc.vector.tensor_add(ic[:], sif[:, 0:CH], ctil[:])
                c_new = cst.tile([B, CH], f32, tag="c")
                nc.vector.tensor_add(c_new[:], fc[:], ic[:])

                # o = sigmoid(acc_o + peep_o * c_new); h = o * tanh(c_new)
                po = gt.tile([B, CH], f32, tag="po")
                nc.vector.tensor_mul(po[:], c_new[:], peep[:, 2, :])
                preo = gt.tile([B, CH], f32, tag="preo")
                nc.vector.tensor_add(preo[:], acc[:, 3 * CH:4 * CH], po[:])
                og = gt.tile([B, CH], f32, tag="og")
                nc.scalar.activation(og[:], preo[:], ACT.Sigmoid)
                th = gt.tile([B, CH], f32, tag="th")
                nc.scalar.activation(th[:], c_new[:], ACT.Tanh)
                hsb = hout.tile([B, CH], f16, tag="hsb")
                nc.vector.tensor_mul(hsb[:], og[:], th[:])

                # output chunk for this step (host re-assembles/casts)
                nc.sync.dma_start(out_d[t], hsb[:])

                # transpose h chunk -> [128ch, 64b] and AllGather across cores
                pt_t = ptr.tile([CH, B], f16, tag="pt")
                nc.tensor.transpose(pt_t[:], hsb[:], ident[:])
                htr = hout.tile([CH, B], f16, tag="htr")
                nc.vector.tensor_copy(htr[:], pt_t[:])
                exch = os.environ.get("K_EXCH", "ag")
                gath = os.environ.get("K_GATHER", "one")
                inb = dbb.tile([CH, B], f16, tag="inb")
                nc.sync.dma_start(inb[:], htr[:])
                outb = dbb.tile([H, B], f16, tag="outb")
                if exch == "ag":
                    nc.gpsimd.collective_compute(
                        "AllGather",
                        mybir.AluOpType.bypass,
                        replica_groups=[list(range(NCORES))],
                        ins=[inb[:].opt()],
                        outs=[outb[:].opt()],
                    )
                else:  # diagnostic: fake the gather with 8 local copies
                    for r in range(NCORES):
                        nc.sync.dma_start(outb[r * CH:(r + 1) * CH, :], inb[:])
                hbuf = hga.tile([128, KT, B], f16, tag="hbuf")
                if gath == "one":
                    nc.sync.dma_start(
                        hbuf[:], outb[:].rearrange("(k p) b -> p k b", p=128))
                else:  # split into per-k DMAs (parallel queues)
                    for k in range(KT):
                        nc.sync.dma_start(
                            hbuf[:, k, :], outb[k * 128:(k + 1) * 128, :])

                c_prev = c_new

    nc.compile()
    return nc


_NC_CACHE = None


def kernel(x, h0, c0, Wx, bx, Wh, bh, peep, bgate):
    global LAST_RESULT, _NC_CACHE
    from concourse import bass_utils

    x = np.asarray(x, dtype=np.float32)
    h0 = np.asarray(h0, dtype=np.float32)
    c0 = np.asarray(c0, dtype=np.float32)
    Wx = np.asarray(Wx, dtype=np.float32)
    Wh = np.asarray(Wh, dtype=np.float32)
    bx = np.asarray(bx, dtype=np.float32)
    bh = np.asarray(bh, dtype=np.float32)
    peep = np.asarray(peep, dtype=np.float32)
    bgate = np.asarray(bgate, dtype=np.float32)

    # ---- host-side input prep ----
    # xT slab: (S, 128p, 8k, 64b); element = x[b, t, 128k+p]
    xt = np.ascontiguousarray(
        x.transpose(1, 2, 0).reshape(S, KT, 128, B).transpose(0, 2, 1, 3)
    ).astype(np.float16)
    # h0T image: (128p, 8k, 64b)
    h0t = np.ascontiguousarray(
        h0.T.reshape(KT, 128, B).transpose(1, 0, 2)).astype(np.float16)
    btot = (bx + bh + bgate)  # (4, H)

    in_maps = []
    for j in range(NCORES):
        lo_c, hi_c = j * CH, (j + 1) * CH
        # weight slabs: arr[k, p, (g,ch)] = W[g, j*128+ch, 128k+p]
        wr = np.ascontiguousarray(
            Wh[:, lo_c:hi_c, :].transpose(2, 0, 1).reshape(KT, 128, NF))
        wx_ = np.ascontiguousarray(
            Wx[:, lo_c:hi_c, :].transpose(2, 0, 1).reshape(KT, 128, NF))
        wrh, wrl = _split_w(wr)
        wxh, wxl = _split_w(wx_)
        bias_j = np.ascontiguousarray(
            np.broadcast_to(btot[:, lo_c:hi_c].reshape(NF), (B, NF))
        ).astype(np.float32)
        peep_j = np.ascontiguousarray(
            np.broadcast_to(peep[:, lo_c:hi_c][:, None, :], (3, B, CH))
        ).astype(np.float32)
        in_maps.append({
            "xt": xt, "wrh": wrh, "wrl": wrl, "wxh": wxh, "wxl": wxl,
            "h0t": h0t, "c0c": np.ascontiguousarray(c0[:, lo_c:hi_c]),
            "bias": bias_j, "peep": peep_j,
        })

    if _NC_CACHE is None:
        _NC_CACHE = _build_program()
    nc = _NC_CACHE

    # NTFF hook (antenv.axon_hooks) is absent in some containers; force the
    # plain execute path so kernel() never crashes on the profiling import.
    prev = os.environ.get("BASS_NEVER_TRACE")
    os.environ["BASS_NEVER_TRACE"] = "1"
    try:
        res = bass_utils.run_bass_kernel_spmd(
            nc, in_maps, core_ids=list(range(NCORES)))
    finally:
        if prev is None:
            os.environ.pop("BASS_NEVER_TRACE", None)
        else:
            os.environ["BASS_NEVER_TRACE"] = prev
    LAST_RESULT = res

    # ---- assemble full output: res[j]["out"] is (S, B, CH) fp16 ----
    chunks = [r["out"].astype(np.float32) for r in res.results]  # list of (S,B,CH)
    full = np.stack(chunks, axis=0)          # (8, S, B, CH)
    full = full.transpose(2, 1, 0, 3).reshape(B, S, H)
    return np.ascontiguousarray(full)


if __name__ == "__main__":
    # smoke: build only
    prog = _build_program()
    print("build ok")
